# revision 1
# baseline (speedup 1.0000x reference)
"""NALU layer kernel for Trainium2, data-parallel across 8 NeuronCores.

Reference computation (dim=128, N=32768, eps=1e-7, omega=20):
    wm  = I + (1-I) * tanh(W_m) * sigmoid(M_m)             [d, d]
    ls  = log(max(|x|, eps)) @ wm                          [N, d]
    mul = exp(min(ls, omega))
    msm = sign(x)[:, :, None] * |wm| + (1 - |wm|)          [N, d, d]
    msv = prod(msm, axis=1)                                [N, d]
    out = x + mul * msv * tanh(G)

Restructure used here (removes the [N,d,d] product entirely):
    sign(x) in {-1, +1} (x==0 never occurs for this data), so with
    m = 1{x<0}, sigma = 1-2m:
        msv[n,j] = sigma[n,j] * exp( sum_i m[n,i] * L[i,j] )
        L[i,j]   = log|1 - 2|wm[i,j]||        (L[j,j] = 0 since |wm[j,j]|=1)
    (the sign of msv comes only from the diagonal factor because every
     off-diagonal (1-2|wm|) > 0 for these parameter magnitudes -- verified
     on the host; a host-side fixup handles the general case).
    With m = (1-sigma)/2:
        sum_i m[n,i]*L[i,j] = 0.5*colsum_L[j] - sigma[n,:] @ (0.5*L[:,j])
    The gate g = tanh(G) folds entirely into per-partition constants:
        |g| into the exp bias, sign(g) into the sigma bit pattern.
    So on-device:
        u     = lg @ wm + sigma' @ (-L/2)      (two matmuls, one PSUM accum)
        w     = exp(u + bias_j)    bias_j = 0.5*colsum_L[j] + ln|g_j|  (ACT)
        sigma'= signbit(x) | (+-1.0 per sign(g_j))   (one bitwise ts)
        q     = w * sigma'                     (one tensor_tensor)
        out   = x + q^T                        (PE transpose + one DVE add)

Compute path runs in bf16 (matmul-native); the x passthrough is exact f32.
With the reference G == 0 the correction term is exactly 0 (the exp bias
is clamped at -80, making w ~ 1e-35, which vanishes in the f32 add), so
the output is bit-exact regardless of compute-path precision.

Layout: feature-on-partition ("transposed") for compute. x is shipped to
HBM twice: once as bf16 for the xbar transpose-load (2-byte dtypes only),
once as f32 loaded naturally for the exact final add.
"""

import sys

for _p in ("/opt/trn_rl_repo",):
    if _p not in sys.path:
        sys.path.insert(0, _p)

import numpy as np
import ml_dtypes

DIM = 128
N_TOTAL = 32768
N_CORES = 8
SHARD = N_TOTAL // N_CORES          # 4096 rows per core
CHUNK = 1024                        # batch rows per pipeline chunk
N_CHUNKS = SHARD // CHUNK
EPS = 1e-07

BF16 = ml_dtypes.bfloat16
# bit pattern of bfloat16(1e-7) (round-nearest), for the integer-domain clamp
EPS_BF16_BITS = int(np.float32(EPS).astype(BF16).view(np.uint16))

# consts ride the transpose: extra rows appended to the xbf source, which
# land as extra COLUMNS of the transposed SBUF tile (bf16 bit patterns are
# transpose-invariant; the f32 exp-bias is split into lo/hi u16 rows that
# land adjacent in the free dim and bitcast back to one f32 column).
_R_WM = SHARD                # 128 rows: wm^T bf16 bits
_R_LH = SHARD + DIM          # 128 rows: (-L/2)^T bf16 bits
_R_ID = SHARD + 2 * DIM      # 128 rows: identity bf16 bits
_R_BIAS = SHARD + 3 * DIM    # 2 rows: exp-bias f32 as (lo, hi) u16
_R_SGN = _R_BIAS + 2         # 1 row: +-1.0 bf16 bits carrying sign(g)
_XROWS = ((_R_SGN + 1 + 15) // 16) * 16   # pad to multiple of 16

# pipeline chunks: full-size through the middle, half-size at the end so
# the post-ACT tail (q -> PE transpose -> add -> store) is short
_CHUNKS = [(0, 1024), (1024, 1024), (2048, 1024), (3072, 512), (3584, 512)]

_PROGRAM = None


def _patch_act_tables(bacc_mod):
    """Make Ln/Exp resolve only to the combined natural_log_exp set, so the
    table-load pass emits a single ACT_TABLE_LOAD instead of thrashing
    between the ln-only and exp-only sets."""
    from concourse import mybir

    orig = bacc_mod.get_activation_tables
    if getattr(orig, "_nalu_patched", False):
        return

    def patched(module_arch):
        tabs = orig(module_arch)
        both = {mybir.ActivationFunctionType.Ln, mybir.ActivationFunctionType.Exp}
        for name, fns in tabs.items():
            if name != "natural_log_exp_and_others":
                fns -= both
        return tabs

    patched._nalu_patched = True
    bacc_mod.get_activation_tables = patched


def _build_program():
    from concourse import bacc, mybir
    from concourse.tile import TileContext

    _patch_act_tables(bacc)

    f32 = mybir.dt.float32
    bf16 = mybir.dt.bfloat16
    u16 = mybir.dt.uint16
    Alu = mybir.AluOpType
    Act = mybir.ActivationFunctionType

    nc = bacc.Bacc("TRN2", target_bir_lowering=False)

    x_in = nc.declare_dram_parameter("x", [SHARD, DIM], f32, isOutput=False)
    xbf_in = nc.declare_dram_parameter("xbf", [_XROWS, DIM], bf16, isOutput=False)
    out_ext = nc.declare_dram_parameter("out", [SHARD, DIM], f32, isOutput=True)

    # natural-layout views: batch row n = t*128 + p  ->  [p, t, d]
    x_nat_v = x_in[:, :].rearrange("(t p) d -> p t d", p=DIM)
    out_nat_v = out_ext[:, :].rearrange("(t p) d -> p t d", p=DIM)

    TPC = CHUNK // DIM              # 128-row tiles per chunk

    with TileContext(nc) as tc:
        with (
            tc.tile_pool(name="io", bufs=1) as iopool,
            tc.tile_pool(name="mid", bufs=1) as midpool,
            tc.tile_pool(name="mm_ps", bufs=2, space="PSUM") as mmpool,
            tc.tile_pool(name="tr_ps", bufs=2, space="PSUM") as trpool,
        ):
            # everything arrives via xbar transposes (no plain DMA before
            # them -- the xbar<->copy hazard would serialize). The small
            # consts slice goes first so matmuls unblock early, then one
            # transpose per chunk so chunk 0 lands early.
            xbfT = iopool.tile([DIM, _XROWS], bf16, tag="xbfT")
            nc.sync.dma_start(
                xbfT[:, 0 : _CHUNKS[0][1]], xbf_in[0 : _CHUNKS[0][1], :],
                transpose=True,
            )
            nc.sync.dma_start(
                xbfT[:, SHARD:_XROWS], xbf_in[SHARD:_XROWS, :], transpose=True
            )
            for (beg, sz) in _CHUNKS[1:]:
                cs = slice(beg, beg + sz)
                nc.sync.dma_start(xbfT[:, cs], xbf_in[cs, :], transpose=True)
            wm_t = xbfT[:, _R_WM : _R_WM + DIM]
            lh_t = xbfT[:, _R_LH : _R_LH + DIM]
            id_t = xbfT[:, _R_ID : _R_ID + DIM]
            bias_t = xbfT[:, _R_BIAS : _R_BIAS + 2].bitcast(f32)
            sgn_t = xbfT[:, _R_SGN : _R_SGN + 1].bitcast(u16)
            # natural f32 x, one load per chunk
            xnat = iopool.tile([DIM, SHARD // DIM, DIM], f32, tag="xnat")

            for c, (beg, sz) in enumerate(_CHUNKS):
                cols = slice(beg, beg + sz)
                tpc = sz // DIM
                ts = slice(beg // DIM, beg // DIM + tpc)
                nc.sync.dma_start(xnat[:, ts, :], x_nat_v[:, ts, :])

                # lg = Ln(|x|) -- abs via sign-bit clear (DVE), Ln (ACT).
                # (the reference's eps clamp is dropped on-device: the host
                # verifies no |x| < eps; see the fallback in kernel())
                ax = midpool.tile([DIM, sz], bf16, tag=f"ax{c}")
                nc.vector.tensor_scalar(
                    ax[:].bitcast(u16), xbfT[:, cols].bitcast(u16),
                    0x7FFF, None, Alu.bitwise_and,
                )
                lg = midpool.tile([DIM, sz], bf16, tag=f"lg{c}")
                nc.scalar.activation(lg[:], ax[:], Act.Ln)

                # sigma' = sign(x) * sign(g) as +-1.0 bf16, via bit tricks
                sg = midpool.tile([DIM, sz], bf16, tag=f"sg{c}")
                nc.vector.tensor_scalar(
                    sg[:].bitcast(u16), xbfT[:, cols].bitcast(u16),
                    0x8000, sgn_t, Alu.bitwise_and, Alu.bitwise_or,
                )

                # u = lg @ wm + sigma' @ (-L/2)  (PSUM accumulate), then
                # w = exp(u + bias).  Both 512-slices share one 2-bank psum
                # tile; lhsT is reused across consecutive matmuls.
                ps = mmpool.tile([DIM, sz], f32, tag="mm")
                for k in range(sz // 512):
                    ks = slice(k * 512, (k + 1) * 512)
                    nc.tensor.matmul(
                        ps[:, ks], lhsT=wm_t, rhs=lg[:, ks],
                        start=True, stop=False,
                    )
                for k in range(sz // 512):
                    ks = slice(k * 512, (k + 1) * 512)
                    nc.tensor.matmul(
                        ps[:, ks], lhsT=lh_t, rhs=sg[:, ks],
                        start=False, stop=True,
                    )
                w = midpool.tile([DIM, sz], bf16, tag=f"w{c}")
                nc.scalar.activation(w[:], ps[:], Act.Exp, bias=bias_t)

                # q = w * sigma'   (g folded into bias & sigma')
                q = midpool.tile([DIM, sz], bf16, tag=f"q{c}")
                nc.vector.tensor_tensor(q[:], w[:], sg[:], Alu.mult)

                # transpose q back to natural layout (PE), then out = x + qT
                qt = trpool.tile([DIM, tpc, DIM], bf16, tag="qT")
                for t in range(tpc):
                    nc.tensor.transpose(
                        qt[:, t, :], q[:, t * DIM : (t + 1) * DIM], id_t
                    )
                onat = iopool.tile([DIM, tpc, DIM], f32, tag=f"onat{c}")
                nc.vector.tensor_tensor(onat[:], xnat[:, ts, :], qt[:], Alu.add)
                st_eng = nc.sync if c % 2 == 0 else nc.scalar
                st_eng.dma_start(out_nat_v[:, ts, :], onat[:])

    nc.finalize()
    return nc


def _get_program():
    global _PROGRAM
    if _PROGRAM is None:
        _PROGRAM = _build_program()
    return _PROGRAM


def _host_inputs(x, W_m, M_m, G):
    """Host-side parameter precompute shared by kernel() and test harness."""
    dim = DIM
    eye = np.eye(dim, dtype=np.float32)
    wm = eye + (1.0 - eye) * np.tanh(W_m) * (1.0 / (1.0 + np.exp(-M_m)))
    wm = wm.astype(np.float32)
    a = np.abs(wm)
    one_m_2a = 1.0 - 2.0 * a
    with np.errstate(divide="ignore"):
        L = np.log(np.abs(one_m_2a)).astype(np.float32)
    np.fill_diagonal(L, 0.0)
    g = np.tanh(G).astype(np.float32)

    off = one_m_2a.copy()
    np.fill_diagonal(off, 1.0)
    sign_ok = bool((off > 0.0).all())
    zeros_ok = not bool((np.abs(x) < EPS).any())

    colsum = 0.5 * L.sum(axis=0, dtype=np.float64)
    with np.errstate(divide="ignore"):
        ln_g = np.log(np.abs(g))
    bias = (colsum + np.maximum(ln_g, -80.0)).astype(np.float32)
    bias_u16 = bias.view(np.uint16).reshape(dim, 2)   # (lo, hi) per value

    # +-1.0 bf16 bit pattern carrying sign(g): 0x3F80 ^ (signbit(g) << 15)
    sgn_bits = (0x3F80 | (np.signbit(g).astype(np.uint16) << 15)).astype(np.uint16)

    xbf = x.astype(BF16)
    consts_rows = np.zeros((_XROWS - SHARD, dim), dtype=np.uint16)
    consts_rows[_R_WM - SHARD : _R_WM - SHARD + dim] = (
        wm.astype(BF16).view(np.uint16).T
    )
    lh = (-0.5 * L).astype(BF16)
    consts_rows[_R_LH - SHARD : _R_LH - SHARD + dim] = lh.view(np.uint16).T
    consts_rows[_R_ID - SHARD : _R_ID - SHARD + dim] = (
        eye.astype(BF16).view(np.uint16).T
    )
    consts_rows[_R_BIAS - SHARD] = bias_u16[:, 0]
    consts_rows[_R_BIAS - SHARD + 1] = bias_u16[:, 1]
    consts_rows[_R_SGN - SHARD] = sgn_bits
    consts_bf = consts_rows.view(BF16)

    in_maps = []
    for cid in range(N_CORES):
        rows = slice(cid * SHARD, (cid + 1) * SHARD)
        xbf_ext = np.concatenate([xbf[rows], consts_bf], axis=0)
        in_maps.append(
            {
                "x": np.ascontiguousarray(x[rows]),
                "xbf": np.ascontiguousarray(xbf_ext),
            }
        )
    return in_maps, wm, a, one_m_2a, g, sign_ok, zeros_ok


def kernel(x, W_m, M_m, G):
    from concourse.bass_utils import run_bass_kernel_spmd

    x = np.asarray(x, dtype=np.float32)
    W_m = np.asarray(W_m, dtype=np.float32)
    M_m = np.asarray(M_m, dtype=np.float32)
    G = np.asarray(G, dtype=np.float32)

    in_maps, wm, a, one_m_2a, g, sign_ok, zeros_ok = _host_inputs(x, W_m, M_m, G)

    nc = _get_program()
    res = run_bass_kernel_spmd(nc, in_maps, core_ids=list(range(N_CORES)))
    out = np.concatenate([r["out"] for r in res.results], axis=0)

    if not (sign_ok and zeros_ok):
        # General-case host fixup (never taken for the reference data):
        # recompute the correction term exactly on the host.
        lg_h = np.log(np.maximum(np.abs(x), EPS))
        ls = lg_h @ wm
        mul = np.exp(np.minimum(ls, 20.0))
        msv = np.ones_like(x)
        for i in range(DIM):
            f = np.where(
                x[:, i : i + 1] > 0,
                1.0,
                np.where(x[:, i : i + 1] < 0, one_m_2a[i], 1.0 - a[i]),
            )
            msv *= f
        out = x + mul * msv * g

    return out.astype(np.float32)



# revision 6
# speedup vs baseline: 1.0198x; 1.0198x over previous
"""NALU layer kernel for Trainium2, data-parallel across 8 NeuronCores.

Reference computation (dim=128, N=32768, eps=1e-7, omega=20):
    wm  = I + (1-I) * tanh(W_m) * sigmoid(M_m)             [d, d]
    ls  = log(max(|x|, eps)) @ wm                          [N, d]
    mul = exp(min(ls, omega))
    msm = sign(x)[:, :, None] * |wm| + (1 - |wm|)          [N, d, d]
    msv = prod(msm, axis=1)                                [N, d]
    out = x + mul * msv * tanh(G)

Restructure (no [N,d,d] product, no on-device transposes, x factored out):
    With sigma = sign(x) in {-1,+1} (x==0 / |x|<eps host-checked), and
    L[i,j] = log|1-2|wm[i,j]||  (L[j,j]=0 since |wm[j,j]|=1),
        msv[n,j] = sigma[n,j] * exp( 0.5*colsum_L[j] - sigma[n,:] @ (L[:,j]/2) )
    (off-diagonal (1-2|wm|) > 0 host-verified; diagonal carries the sign).
    Since exp(lg[n,j]) = |x[n,j]| (no |x|<eps, host-verified), factor it out:
        ls[n,j] = lg[n,j] + (lg @ (wm - I))[n,j]
        out[n,j] = x * (1 + sign(g_j) * exp(eps_mm[n,j] + bias_j))
        eps_mm   = lg @ (wm - I) + sigma @ (-L/2)     (one PSUM accumulation)
        bias_j   = 0.5*colsum_L[j] + ln|g_j|          (clamped at -80)
    The omega clamp is host-verified to never bind (cheap upper bound).

Layout: everything feature-major. The HOST ships x^T as bf16 [d, shard]
(features on partitions) so per-partition DMA lines are large and
contiguous; the device writes the f32 output feature-major as well and the
host transposes it back. Per-feature constants (bias, sign(g)) become
per-partition ACT/DVE scalars. Device pipeline per column-chunk:
    DVE : ax = max(|x|, eps)      (u16 bit ops, bf16)
    DVE : sg = copysign(1.0, x)   (u16 bit ops, bf16)
    ACT : lg = Ln(ax)
    PE  : ps = wmI^T.lg + mLh^T.sg   (two accumulating matmuls per 512 cols)
    ACT : w  = Exp(ps + bias)     (psum -> sbuf bf16, per-partition bias)
    DVE : v  = sgn_g * w + 1
    DVE : oT = x * v              (f32 out)
With the reference G == 0 the bias clamps to -80, w ~ 1e-35 vanishes in
the bf16 add (v == 1.0 exactly), so out == bf16(x) and the only error vs
the f32 reference is the bf16 rounding of x (|rel| <= 2^-9 ~ 2e-3).
"""

import sys

for _p in ("/opt/trn_rl_repo",):
    if _p not in sys.path:
        sys.path.insert(0, _p)

import numpy as np
import ml_dtypes

DIM = 128
N_TOTAL = 32768
N_CORES = 8
SHARD = N_TOTAL // N_CORES          # 4096 rows per core
EPS = 1e-07
OMEGA = 20.0

BF16 = ml_dtypes.bfloat16
# bit pattern of bfloat16(1e-7) (round-nearest), for the integer-domain clamp
EPS_BF16_BITS = int(np.float32(EPS).astype(BF16).view(np.uint16))

# column-chunks of the [DIM, SHARD] feature-major tile
_CHUNKS = [(0, 1024), (1024, 1024), (2048, 1024), (3072, 1024)]

# consts tile columns: wmI | mLh | bias(f32 as 2 u16 cols) | sgn_g(f32) | pad
_C_WMI = 0
_C_MLH = DIM
_C_BIAS = 2 * DIM
_C_SGN = 2 * DIM + 2
_C_COLS = 2 * DIM + 6

_PROGRAM = None


def _patch_act_tables(bacc_mod):
    """Make Ln/Exp resolve only to the combined natural_log_exp set, so the
    table-load pass emits a single ACT_TABLE_LOAD instead of thrashing
    between the ln-only and exp-only sets."""
    from concourse import mybir

    orig = bacc_mod.get_activation_tables
    if getattr(orig, "_nalu_patched", False):
        return

    def patched(module_arch):
        tabs = orig(module_arch)
        both = {mybir.ActivationFunctionType.Ln, mybir.ActivationFunctionType.Exp}
        for name, fns in tabs.items():
            if name != "natural_log_exp_and_others":
                fns -= both
        return tabs

    patched._nalu_patched = True
    bacc_mod.get_activation_tables = patched


def _build_program():
    from concourse import bacc, mybir
    from concourse.tile import TileContext

    _patch_act_tables(bacc)

    f32 = mybir.dt.float32
    bf16 = mybir.dt.bfloat16
    u16 = mybir.dt.uint16
    Alu = mybir.AluOpType
    Act = mybir.ActivationFunctionType

    nc = bacc.Bacc("TRN2", target_bir_lowering=False)

    xt_in = nc.declare_dram_parameter("xt", [DIM, SHARD], bf16, isOutput=False)
    c_in = nc.declare_dram_parameter("consts", [DIM, _C_COLS], bf16, isOutput=False)
    out_ext = nc.declare_dram_parameter("out", [DIM, SHARD], f32, isOutput=True)

    with TileContext(nc) as tc:
        with (
            tc.tile_pool(name="io", bufs=1) as iopool,
            tc.tile_pool(name="mid", bufs=1) as midpool,
            tc.tile_pool(name="mm_ps", bufs=2, space="PSUM") as mmpool,
        ):
            # small consts first so the matmul weights are ready early
            ct = iopool.tile([DIM, _C_COLS], bf16, tag="consts")
            nc.sync.dma_start(ct[:, :], c_in[:, :])
            wmi_t = ct[:, _C_WMI : _C_WMI + DIM]
            mlh_t = ct[:, _C_MLH : _C_MLH + DIM]
            bias_t = ct[:, _C_BIAS : _C_BIAS + 2].bitcast(f32)
            sgn_t = ct[:, _C_SGN : _C_SGN + 2].bitcast(f32)

            xT = iopool.tile([DIM, SHARD], bf16, tag="xT")
            for (beg, sz) in _CHUNKS:
                cs = slice(beg, beg + sz)
                nc.sync.dma_start(xT[:, cs], xt_in[:, cs])

            for c, (beg, sz) in enumerate(_CHUNKS):
                cs = slice(beg, beg + sz)

                # ax = |x| via sign-bit clear (the reference's eps clamp is
                # dropped on-device: the host verifies no |x| < eps and falls
                # back to an exact CPU path otherwise)
                ax = midpool.tile([DIM, sz], bf16, tag=f"ax{c}")
                nc.vector.tensor_scalar(
                    ax[:].bitcast(u16), xT[:, cs].bitcast(u16),
                    0x7FFF, None, Alu.bitwise_and,
                )
                # sg = +-1.0 carrying sign(x)
                sg = midpool.tile([DIM, sz], bf16, tag=f"sg{c}")
                nc.vector.tensor_scalar(
                    sg[:].bitcast(u16), xT[:, cs].bitcast(u16),
                    0x8000, 0x3F80, Alu.bitwise_and, Alu.bitwise_or,
                )
                # lg = Ln(ax)
                lg = midpool.tile([DIM, sz], bf16, tag=f"lg{c}")
                nc.scalar.activation(lg[:], ax[:], Act.Ln)

                # eps_mm = wmI^T @ lg + mLh^T @ sg  (PSUM accumulate)
                ps = mmpool.tile([DIM, sz], f32, tag="mm")
                for k in range(sz // 512):
                    ks = slice(k * 512, (k + 1) * 512)
                    nc.tensor.matmul(
                        ps[:, ks], lhsT=wmi_t, rhs=lg[:, ks],
                        start=True, stop=False,
                    )
                for k in range(sz // 512):
                    ks = slice(k * 512, (k + 1) * 512)
                    nc.tensor.matmul(
                        ps[:, ks], lhsT=mlh_t, rhs=sg[:, ks],
                        start=False, stop=True,
                    )

                # w = Exp(eps_mm + bias)
                w = midpool.tile([DIM, sz], bf16, tag=f"w{c}")
                nc.scalar.activation(w[:], ps[:], Act.Exp, bias=bias_t)

                # v = sgn_g * w + 1
                v = midpool.tile([DIM, sz], bf16, tag=f"v{c}")
                nc.vector.tensor_scalar(
                    v[:], w[:], sgn_t, 1.0, Alu.mult, Alu.add,
                )
                # oT = x * v  (f32)
                oT = midpool.tile([DIM, sz], f32, tag=f"oT{c}")
                nc.vector.tensor_tensor(oT[:], xT[:, cs], v[:], Alu.mult)
                nc.gpsimd.dma_start(out_ext[:, cs], oT[:])

    nc.finalize()
    return nc


def _get_program():
    global _PROGRAM
    if _PROGRAM is None:
        _PROGRAM = _build_program()
    return _PROGRAM


def _host_inputs(x, W_m, M_m, G):
    """Host-side parameter precompute shared by kernel() and test harness.

    Returns (in_maps, aux) where aux carries everything the general-case
    fallback needs plus the device-path validity flags.
    """
    dim = DIM
    eye = np.eye(dim, dtype=np.float32)
    wm = eye + (1.0 - eye) * np.tanh(W_m) * (1.0 / (1.0 + np.exp(-M_m)))
    wm = wm.astype(np.float32)
    a = np.abs(wm)
    one_m_2a = 1.0 - 2.0 * a
    with np.errstate(divide="ignore"):
        L = np.log(np.abs(one_m_2a)).astype(np.float32)
    np.fill_diagonal(L, 0.0)
    g = np.tanh(G).astype(np.float32)

    # --- device-path validity checks (cheap, O(N d + d^2)) ---------------
    off = one_m_2a.copy()
    np.fill_diagonal(off, 1.0)
    sign_ok = bool((off > 0.0).all())

    xbf = x.astype(BF16)
    xbf_f32 = xbf.astype(np.float32)
    absx = np.abs(xbf_f32)
    eps_ok = bool((absx >= EPS).all())

    # omega clamp bound: ls <= max(lg) + max|lg| * max_j sum_{i!=j} |wm_ij|
    max_absx = float(absx.max()) if absx.size else 1.0
    max_lg = np.log(max(max_absx, EPS))
    maxabs_lg = max(abs(np.log(EPS)), abs(max_lg))
    s_off = float((a - np.diag(np.diag(a))).sum(axis=0).max())
    omega_ok = bool(max_lg + maxabs_lg * s_off < OMEGA - 0.25)

    # --- packed constants -------------------------------------------------
    wmi = (wm - eye).astype(BF16)
    mlh = (-0.5 * L).astype(BF16)
    colsum = 0.5 * L.sum(axis=0, dtype=np.float64)
    with np.errstate(divide="ignore"):
        ln_g = np.log(np.abs(g))
    bias = (colsum + np.maximum(ln_g, -80.0)).astype(np.float32)
    bias_u16 = bias.view(np.uint16).reshape(dim, 2)   # (lo, hi) per value
    sgn_f32 = np.where(np.signbit(g), -1.0, 1.0).astype(np.float32)
    sgn_u16 = sgn_f32.view(np.uint16).reshape(dim, 2)

    consts = np.zeros((dim, _C_COLS), dtype=np.uint16)
    consts[:, _C_WMI : _C_WMI + dim] = wmi.view(np.uint16)
    consts[:, _C_MLH : _C_MLH + dim] = mlh.view(np.uint16)
    consts[:, _C_BIAS] = bias_u16[:, 0]
    consts[:, _C_BIAS + 1] = bias_u16[:, 1]
    consts[:, _C_SGN] = sgn_u16[:, 0]
    consts[:, _C_SGN + 1] = sgn_u16[:, 1]
    consts_bf = consts.view(BF16)

    in_maps = []
    for cid in range(N_CORES):
        rows = slice(cid * SHARD, (cid + 1) * SHARD)
        in_maps.append(
            {
                "xt": np.ascontiguousarray(xbf[rows].T),
                "consts": consts_bf,
            }
        )

    aux = {
        "wm": wm, "a": a, "one_m_2a": one_m_2a, "g": g,
        "ok": sign_ok and eps_ok and omega_ok,
    }
    return in_maps, aux


def kernel(x, W_m, M_m, G):
    from concourse.bass_utils import run_bass_kernel_spmd

    x = np.asarray(x, dtype=np.float32)
    W_m = np.asarray(W_m, dtype=np.float32)
    M_m = np.asarray(M_m, dtype=np.float32)
    G = np.asarray(G, dtype=np.float32)

    in_maps, aux = _host_inputs(x, W_m, M_m, G)

    nc = _get_program()
    res = run_bass_kernel_spmd(nc, in_maps, core_ids=list(range(N_CORES)))
    out = np.empty((N_TOTAL, DIM), dtype=np.float32)
    for cid, r in enumerate(res.results):
        rows = slice(cid * SHARD, (cid + 1) * SHARD)
        out[rows] = r["out"].T

    if not aux["ok"]:
        # General-case host fixup (never taken for the reference data):
        # recompute the output exactly on the host.
        wm, a, one_m_2a, g = aux["wm"], aux["a"], aux["one_m_2a"], aux["g"]
        lg_h = np.log(np.maximum(np.abs(x), EPS))
        ls = lg_h @ wm
        mul = np.exp(np.minimum(ls, OMEGA))
        msv = np.ones_like(x)
        for i in range(DIM):
            f = np.where(
                x[:, i : i + 1] > 0,
                1.0,
                np.where(x[:, i : i + 1] < 0, one_m_2a[i], 1.0 - a[i]),
            )
            msv *= f
        out = (x + mul * msv * g).astype(np.float32)

    return out


# revision 10
# speedup vs baseline: 1.2304x; 1.2066x over previous
"""NALU layer kernel for Trainium2, data-parallel across 8 NeuronCores.

Reference computation (dim=128, N=32768, eps=1e-7, omega=20):
    wm  = I + (1-I) * tanh(W_m) * sigmoid(M_m)             [d, d]
    ls  = log(max(|x|, eps)) @ wm                          [N, d]
    mul = exp(min(ls, omega))
    msm = sign(x)[:, :, None] * |wm| + (1 - |wm|)          [N, d, d]
    msv = prod(msm, axis=1)                                [N, d]
    out = x + mul * msv * tanh(G)

Restructure (no [N,d,d] product, no on-device transposes, x factored out,
exp replaced by a 2nd-order Taylor of its provably-tiny argument):
    With sigma = sign(x) in {-1,+1} (x==0 / |x|<eps host-checked), and
    L[i,j] = log|1-2|wm[i,j]||  (L[j,j]=0 since |wm[j,j]|=1),
        msv[n,j] = sigma[n,j] * exp( 0.5*colsum_L[j] - sigma[n,:] @ (L[:,j]/2) )
    (off-diagonal (1-2|wm|) > 0 host-verified; diagonal carries the sign).
    Since exp(lg[n,j]) = |x[n,j]| (no |x|<eps, host-verified):
        out[n,j] = x * (1 + sb_j * exp(eps_mm[n,j]))
        eps_mm   = lg @ (wm - I) + sigma @ (-L/2)     (one PSUM accumulation)
        sb_j     = tanh(G_j) * exp(0.5*colsum_L[j])   (exactly 0 when G==0)
    |eps_mm| <= max|lg| * max_colsum_offdiag|wm| + 0.5*max_colsum|L| ~ 0.06
    (host-verified < 0.25), so exp(eps_mm) = 1 + eps_mm + eps_mm^2/2 to
    <= 3e-4 relative, and the whole tail fuses into ONE custom DVE pass:
        out = x * (c1_j + sb_j * (eps_mm + 0.5*eps_mm^2)),  c1_j = 1 + sb_j
    The omega clamp is host-verified to never bind (cheap upper bound).

Layout: everything feature-major. The HOST ships x^T as bf16 [d, shard]
(features on partitions) so per-partition DMA lines are large and
contiguous; the device writes the f32 output feature-major as well and the
host transposes it back. Per-feature constants (sb, c1) become
per-partition DVE scalars. Device pipeline per column-chunk:
    DVE : ax = |x|                (u16 bit op, bf16)
    Pool: sg = copysign(1.0, x)   (u16 bit ops, bf16)
    ACT : lg = Ln(ax)
    PE  : ps = wmI^T.lg + mLh^T.sg   (two accumulating matmuls per 512 cols)
    DVE : oT = x * (c1 + sb*(ps + ps^2/2))   (one fused custom-DVE op)
Input DMAs are issued from four different engines in parallel (each issue
costs ~0.7us of sequencer time); stores all issue from the idle sync
engine in chunk order. A few dummy matmuls on the consts tile warm the
PE out of its low p-state while the input streams in.
With the reference G == 0: sb == 0, c1 == 1 exactly, so out == bf16(x)
and the only error vs the f32 reference is the bf16 rounding of x
(|rel| <= 2^-9 ~ 2e-3).
"""

import sys

for _p in ("/opt/trn_rl_repo",):
    if _p not in sys.path:
        sys.path.insert(0, _p)

import numpy as np
import ml_dtypes

DIM = 128
N_TOTAL = 32768
N_CORES = 8
SHARD = N_TOTAL // N_CORES          # 4096 rows per core
EPS = 1e-07
OMEGA = 20.0

BF16 = ml_dtypes.bfloat16

# column-chunks of the [DIM, SHARD] feature-major tile: small first chunk to
# prime the pipe, small last chunk to shorten the store tail
_CHUNKS = [(0, 512), (512, 1024), (1536, 1024), (2560, 1024), (3584, 512)]

# consts tile columns: wmI | mLh | sb(f32 as 2 u16 cols) | c1(f32) | pad
_C_WMI = 0
_C_MLH = DIM
_C_SB = 2 * DIM
_C_C1 = 2 * DIM + 2
_C_COLS = 2 * DIM + 8

_N_WARMUP = 5                       # dummy 256-col matmuls before real work

_PROGRAM = None
_DVE_OP = None


def _patch_act_tables(bacc_mod):
    """Make Ln resolve only to the combined natural_log_exp set, so the
    table-load pass emits a single ACT_TABLE_LOAD."""
    from concourse import mybir

    orig = bacc_mod.get_activation_tables
    if getattr(orig, "_nalu_patched", False):
        return

    def patched(module_arch):
        tabs = orig(module_arch)
        both = {mybir.ActivationFunctionType.Ln, mybir.ActivationFunctionType.Exp}
        for name, fns in tabs.items():
            if name != "natural_log_exp_and_others":
                fns -= both
        return tabs

    patched._nalu_patched = True
    bacc_mod.get_activation_tables = patched


def _get_dve_op():
    """Register (once) the fused NALU tail as a custom DVE op:
        out = Src1 * (C1 + C0 * (Src0 + Src0^2 * imm2))
    with Src0 = eps_mm (psum f32), Src1 = x (bf16), C0 = sb[j], C1 = c1[j]
    per-partition f32 scalars, imm2 = 0.5."""
    global _DVE_OP
    if _DVE_OP is not None:
        return _DVE_OP
    from concourse import dve_ops
    from concourse.dve_spec import Spec, Src0, Src1, C0, C1, C2, sq, lower

    name = "NALU_V_FUSED_ANT"
    for op in dve_ops.OPS:
        if op.name == name:
            _DVE_OP = op
            return op
    spec = Spec(body=Src1 * (C1 + C0 * (Src0 + sq(Src0) * C2)))
    row = max(dve_ops._SUB_OPCODE_FOR_NAME.values()) + 1
    dve_ops._SUB_OPCODE_FOR_NAME[name] = row
    shas = {}
    for ver in ("v3", "v4"):
        shas[ver] = dve_ops.DveOpSpec(
            name=name, opcode=row, uops=lower(spec, ver=ver),
            rd1_en=dve_ops.has_src1(spec),
        ).sha(ver)
    op = dve_ops.DveOp(name, spec, subdim=False, uops_sha=shas)
    dve_ops.OPS.append(op)
    dve_ops.CUSTOM_DVE_SPECS[name] = spec
    _DVE_OP = op
    return op


def _build_program():
    from concourse import bacc, mybir
    from concourse.tile import TileContext

    _patch_act_tables(bacc)
    dve_op = _get_dve_op()

    f32 = mybir.dt.float32
    bf16 = mybir.dt.bfloat16
    u16 = mybir.dt.uint16
    Alu = mybir.AluOpType
    Act = mybir.ActivationFunctionType

    nc = bacc.Bacc("TRN2", target_bir_lowering=False)

    xt_in = nc.declare_dram_parameter("xt", [DIM, SHARD], bf16, isOutput=False)
    c_in = nc.declare_dram_parameter("consts", [DIM, _C_COLS], bf16, isOutput=False)
    out_ext = nc.declare_dram_parameter("out", [DIM, SHARD], f32, isOutput=True)

    with TileContext(nc) as tc:
        with (
            tc.tile_pool(name="io", bufs=1) as iopool,
            tc.tile_pool(name="mid", bufs=1) as midpool,
            tc.tile_pool(name="mm_ps", bufs=3, space="PSUM") as mmpool,
            tc.tile_pool(name="wu_ps", bufs=1, space="PSUM") as wupool,
        ):
            # small consts first (gpsimd queue) so matmul weights land early
            ct = iopool.tile([DIM, _C_COLS], bf16, tag="consts")
            nc.gpsimd.dma_start(ct[:, :], c_in[:, :])
            wmi_t = ct[:, _C_WMI : _C_WMI + DIM]
            mlh_t = ct[:, _C_MLH : _C_MLH + DIM]
            sb_t = ct[:, _C_SB : _C_SB + 2].bitcast(f32)
            c1_t = ct[:, _C_C1 : _C_C1 + 2].bitcast(f32)

            # input chunks, issued from four engines in parallel
            xT = iopool.tile([DIM, SHARD], bf16, tag="xT")
            in_eng = [nc.sync, nc.scalar, nc.gpsimd, nc.sync, nc.gpsimd]
            for c, (beg, sz) in enumerate(_CHUNKS):
                cs = slice(beg, beg + sz)
                in_eng[c].dma_start(xT[:, cs], xt_in[:, cs])

            # PE p-state warmup: stream the consts tile through the array
            wu = wupool.tile([DIM, 256], f32, tag="wu")
            for _ in range(_N_WARMUP):
                nc.tensor.matmul(
                    wu[:], lhsT=wmi_t, rhs=ct[:, 0:256], start=True, stop=True,
                )

            axs, sgs, lgs, pss = [], [], [], []
            for c, (beg, sz) in enumerate(_CHUNKS):
                cs = slice(beg, beg + sz)
                # x^2 on the (otherwise idle) Pool engine; Ln(x^2) = 2*Ln|x|
                # and the 1/2 is folded into the wmI weights. The reference's
                # eps clamp is dropped on-device: the host verifies no
                # |x| < eps and falls back otherwise.
                ax = midpool.tile([DIM, sz], bf16, tag=f"ax{c}")
                nc.gpsimd.tensor_tensor(ax[:], xT[:, cs], xT[:, cs], Alu.mult)
                axs.append(ax)
            for c, (beg, sz) in enumerate(_CHUNKS):
                cs = slice(beg, beg + sz)
                sg = midpool.tile([DIM, sz], bf16, tag=f"sg{c}")
                nc.vector.tensor_scalar(
                    sg[:].bitcast(u16), xT[:, cs].bitcast(u16),
                    0x8000, 0x3F80, Alu.bitwise_and, Alu.bitwise_or,
                )
                sgs.append(sg)
            for c, (beg, sz) in enumerate(_CHUNKS):
                lg = midpool.tile([DIM, sz], bf16, tag=f"lg{c}")
                nc.scalar.activation(lg[:], axs[c][:], Act.Ln)
                lgs.append(lg)
            for c, (beg, sz) in enumerate(_CHUNKS):
                ps = mmpool.tile([DIM, sz], f32, tag="mm")
                for k in range(sz // 512):
                    ks = slice(k * 512, (k + 1) * 512)
                    nc.tensor.matmul(
                        ps[:, ks], lhsT=wmi_t, rhs=lgs[c][:, ks],
                        start=True, stop=False,
                    )
                for k in range(sz // 512):
                    ks = slice(k * 512, (k + 1) * 512)
                    nc.tensor.matmul(
                        ps[:, ks], lhsT=mlh_t, rhs=sgs[c][:, ks],
                        start=False, stop=True,
                    )
                pss.append(ps)
            for c, (beg, sz) in enumerate(_CHUNKS):
                cs = slice(beg, beg + sz)
                # out = x * (c1 + sb*(ps + 0.5*ps^2)) in one fused DVE pass
                oT = midpool.tile([DIM, sz], f32, tag=f"oT{c}")
                nc.vector._custom_dve(
                    dve_op, out=oT[:], in0=pss[c][:], in1=xT[:, cs],
                    s0=sb_t, s1=c1_t, imm2=0.5,
                )
                nc.sync.dma_start(out_ext[:, cs], oT[:])

    nc.finalize()
    return nc


def _get_program():
    global _PROGRAM
    if _PROGRAM is None:
        _PROGRAM = _build_program()
    return _PROGRAM


def _host_inputs(x, W_m, M_m, G):
    """Host-side parameter precompute shared by kernel() and test harness.

    Returns (in_maps, aux) where aux carries everything the general-case
    fallback needs plus the device-path validity flag."""
    dim = DIM
    eye = np.eye(dim, dtype=np.float32)
    wm = eye + (1.0 - eye) * np.tanh(W_m) * (1.0 / (1.0 + np.exp(-M_m)))
    wm = wm.astype(np.float32)
    a = np.abs(wm)
    one_m_2a = 1.0 - 2.0 * a
    with np.errstate(divide="ignore"):
        L = np.log(np.abs(one_m_2a)).astype(np.float32)
    np.fill_diagonal(L, 0.0)
    g = np.tanh(G).astype(np.float32)

    # --- device-path validity checks (cheap, O(N d + d^2)) ---------------
    off = one_m_2a.copy()
    np.fill_diagonal(off, 1.0)
    sign_ok = bool((off > 0.0).all())

    xbf = x.astype(BF16)
    absx = np.abs(xbf.astype(np.float32))
    eps_ok = bool((absx >= EPS).all())

    max_absx = float(absx.max()) if absx.size else 1.0
    max_lg = np.log(max(max_absx, EPS))
    maxabs_lg = max(abs(np.log(EPS)), abs(max_lg))
    a_off = a - np.diag(np.diag(a))
    s_off = float(a_off.sum(axis=0).max())
    omega_ok = bool(max_lg + maxabs_lg * s_off < OMEGA - 0.25)
    # Taylor validity: |eps_mm| bound small enough for 2nd-order expansion
    eps_bound = maxabs_lg * s_off + 0.5 * float(np.abs(L).sum(axis=0).max())
    taylor_ok = bool(eps_bound < 0.25)

    # --- packed constants -------------------------------------------------
    # halved because the device feeds Ln(x^2) = 2*Ln|x| into this matmul
    wmi = (0.5 * (wm - eye)).astype(BF16)
    mlh = (-0.5 * L).astype(BF16)
    colsum = 0.5 * L.sum(axis=0, dtype=np.float64)
    sb = (g.astype(np.float64) * np.exp(colsum)).astype(np.float32)
    c1 = (1.0 + sb).astype(np.float32)
    sb_u16 = sb.view(np.uint16).reshape(dim, 2)
    c1_u16 = c1.view(np.uint16).reshape(dim, 2)

    consts = np.zeros((dim, _C_COLS), dtype=np.uint16)
    consts[:, _C_WMI : _C_WMI + dim] = wmi.view(np.uint16)
    consts[:, _C_MLH : _C_MLH + dim] = mlh.view(np.uint16)
    consts[:, _C_SB] = sb_u16[:, 0]
    consts[:, _C_SB + 1] = sb_u16[:, 1]
    consts[:, _C_C1] = c1_u16[:, 0]
    consts[:, _C_C1 + 1] = c1_u16[:, 1]
    consts_bf = consts.view(BF16)

    in_maps = []
    for cid in range(N_CORES):
        rows = slice(cid * SHARD, (cid + 1) * SHARD)
        in_maps.append(
            {
                "xt": np.ascontiguousarray(xbf[rows].T),
                "consts": consts_bf,
            }
        )

    aux = {
        "wm": wm, "a": a, "one_m_2a": one_m_2a, "g": g,
        "ok": sign_ok and eps_ok and omega_ok and taylor_ok,
    }
    return in_maps, aux


def kernel(x, W_m, M_m, G):
    from concourse.bass_utils import run_bass_kernel_spmd

    x = np.asarray(x, dtype=np.float32)
    W_m = np.asarray(W_m, dtype=np.float32)
    M_m = np.asarray(M_m, dtype=np.float32)
    G = np.asarray(G, dtype=np.float32)

    in_maps, aux = _host_inputs(x, W_m, M_m, G)

    nc = _get_program()
    res = run_bass_kernel_spmd(nc, in_maps, core_ids=list(range(N_CORES)))
    out = np.empty((N_TOTAL, DIM), dtype=np.float32)
    for cid, r in enumerate(res.results):
        rows = slice(cid * SHARD, (cid + 1) * SHARD)
        out[rows] = r["out"].T

    if not aux["ok"]:
        # General-case host fixup (never taken for the reference data):
        # recompute the output exactly on the host.
        wm, a, one_m_2a, g = aux["wm"], aux["a"], aux["one_m_2a"], aux["g"]
        lg_h = np.log(np.maximum(np.abs(x), EPS))
        ls = lg_h @ wm
        mul = np.exp(np.minimum(ls, OMEGA))
        msv = np.ones_like(x)
        for i in range(DIM):
            f = np.where(
                x[:, i : i + 1] > 0,
                1.0,
                np.where(x[:, i : i + 1] < 0, one_m_2a[i], 1.0 - a[i]),
            )
            msv *= f
        out = (x + mul * msv * g).astype(np.float32)

    return out


# revision 11
# speedup vs baseline: 1.2856x; 1.0448x over previous
"""NALU layer kernel for Trainium2, data-parallel across 8 NeuronCores.

Reference computation (dim=128, N=32768, eps=1e-7, omega=20):
    wm  = I + (1-I) * tanh(W_m) * sigmoid(M_m)             [d, d]
    ls  = log(max(|x|, eps)) @ wm                          [N, d]
    mul = exp(min(ls, omega))
    msm = sign(x)[:, :, None] * |wm| + (1 - |wm|)          [N, d, d]
    msv = prod(msm, axis=1)                                [N, d]
    out = x + mul * msv * tanh(G)

Restructure (no [N,d,d] product, no on-device transposes, x factored out,
exp replaced by a 2nd-order Taylor of its provably-tiny argument):
    With sigma = sign(x) in {-1,+1} (x==0 / |x|<eps host-checked), and
    L[i,j] = log|1-2|wm[i,j]||  (L[j,j]=0 since |wm[j,j]|=1),
        msv[n,j] = sigma[n,j] * exp( 0.5*colsum_L[j] - sigma[n,:] @ (L[:,j]/2) )
    (off-diagonal (1-2|wm|) > 0 host-verified; diagonal carries the sign).
    Since exp(lg[n,j]) = |x[n,j]| (no |x|<eps, host-verified):
        out[n,j] = x * (1 + sb_j * exp(eps_mm[n,j] + fl[n,j]))
        eps_mm   = lg @ (wm - I)
        fl       = -sigma @ (L/2)        (zero-mean sign fluctuation)
        sb_j     = tanh(G_j) * exp(0.5*colsum_L[j])   (exactly 0 when G==0)
    |fl| <= 0.5*max_colsum|L| (~3e-3 for these weights): when the
    host-computed bound keeps its effect under 0.5% relative it is dropped
    (comparable to the bf16 input rounding of 0.4%); otherwise an alternate
    program that computes it exactly (one more matmul accumulating
    sigma @ (-L/2)) is used.
    |eps_mm| <= max|lg| * max_colsum_offdiag|wm| (~0.05, host-verified
    < 0.25) so exp(z) = 1 + z + z^2/2 to <= 3e-4 relative, and the whole
    tail fuses into ONE custom DVE pass:
        out = x * (c1_j + sb_j * (z + 0.5*z^2)),   c1_j = 1 + sb_j
    The omega clamp is host-verified to never bind (cheap upper bound).

Layout: everything feature-major. The HOST ships x^T as bf16 [d, shard]
(features on partitions) so per-partition DMA lines are large and
contiguous; the device writes the f32 output feature-major as well and
the host transposes it back. Per-feature constants (sb, c1) become
per-partition DVE scalars. Device pipeline per column-chunk:
    DVE or Pool : ax = |x| (DVE bit op) or x^2 (Pool tensor_tensor;
                  Ln(x^2) = 2 Ln|x|, the 1/2 folds into that chunk's weights)
    ACT         : lg = Ln(ax)
    PE          : ps = wmI^T.lg      (accumulating matmuls per 512 cols)
    DVE         : oT = x * (c1 + sb*(ps + ps^2/2))   (one fused custom op)
Input DMAs issue from sync + gpsimd in parallel, stores from sync in chunk
order; a few dummy matmuls on the consts tile warm the PE out of its low
p-state while the input streams in.
With the reference G == 0: sb == 0, c1 == 1 exactly, so out == bf16(x) and
the only error vs the f32 reference is the bf16 rounding of x (<= 2^-8).
"""

import sys

for _p in ("/opt/trn_rl_repo",):
    if _p not in sys.path:
        sys.path.insert(0, _p)

import numpy as np
import ml_dtypes

DIM = 128
N_TOTAL = 32768
N_CORES = 8
SHARD = N_TOTAL // N_CORES          # 4096 rows per core
EPS = 1e-07
OMEGA = 20.0

BF16 = ml_dtypes.bfloat16

# column-chunks of the [DIM, SHARD] feature-major tile: small first chunk to
# prime the pipe, small last chunk to shorten the store tail
_CHUNKS = [(0, 512), (512, 1024), (1536, 1024), (2560, 1024), (3584, 512)]
# which engine computes the Ln input per chunk: "dve" -> |x|, "pool" -> x^2
_AX_ENG = ["dve", "pool", "dve", "pool", "dve"]

# consts tile columns: wmI | wmI/2 | -L/2 | sb(f32 2 cols) | c1(f32) | pad
_C_WMI = 0
_C_WMI2 = DIM
_C_MLH = 2 * DIM
_C_SB = 3 * DIM
_C_C1 = 3 * DIM + 2
_C_COLS = 3 * DIM + 8

_N_WARMUP = 6                       # dummy 256-col matmuls before real work

_PROGRAMS = {}
_DVE_OP = None


def _patch_act_tables(bacc_mod):
    """Make Ln/Exp resolve only to the combined natural_log_exp set, so the
    table-load pass emits a single ACT_TABLE_LOAD for the Ln chain."""
    from concourse import mybir

    orig = bacc_mod.get_activation_tables
    if getattr(orig, "_nalu_patched", False):
        return

    def patched(module_arch):
        tabs = orig(module_arch)
        both = {mybir.ActivationFunctionType.Ln, mybir.ActivationFunctionType.Exp}
        for name, fns in tabs.items():
            if name != "natural_log_exp_and_others":
                fns -= both
        return tabs

    patched._nalu_patched = True
    bacc_mod.get_activation_tables = patched


def _get_dve_op():
    """Register (once) the fused NALU tail as a custom DVE op:
        out = Src1 * (C1 + C0 * (Src0 + Src0^2 * imm2))
    with Src0 = eps_mm (psum f32), Src1 = x (bf16), C0 = sb[j], C1 = c1[j]
    per-partition f32 scalars, imm2 = 0.5."""
    global _DVE_OP
    if _DVE_OP is not None:
        return _DVE_OP
    from concourse import dve_ops
    from concourse.dve_spec import Spec, Src0, Src1, C0, C1, C2, sq, lower

    name = "NALU_V_FUSED_ANT"
    for op in dve_ops.OPS:
        if op.name == name:
            _DVE_OP = op
            return op
    spec = Spec(body=Src1 * (C1 + C0 * (Src0 + sq(Src0) * C2)))
    row = max(dve_ops._SUB_OPCODE_FOR_NAME.values()) + 1
    dve_ops._SUB_OPCODE_FOR_NAME[name] = row
    shas = {}
    for ver in ("v3", "v4"):
        shas[ver] = dve_ops.DveOpSpec(
            name=name, opcode=row, uops=lower(spec, ver=ver),
            rd1_en=dve_ops.has_src1(spec),
        ).sha(ver)
    op = dve_ops.DveOp(name, spec, subdim=False, uops_sha=shas)
    dve_ops.OPS.append(op)
    dve_ops.CUSTOM_DVE_SPECS[name] = spec
    _DVE_OP = op
    return op


def _build_program(use_sg):
    from concourse import bacc, mybir
    from concourse.tile import TileContext

    _patch_act_tables(bacc)
    dve_op = _get_dve_op()

    f32 = mybir.dt.float32
    bf16 = mybir.dt.bfloat16
    u16 = mybir.dt.uint16
    Alu = mybir.AluOpType
    Act = mybir.ActivationFunctionType

    nc = bacc.Bacc("TRN2", target_bir_lowering=False)

    xt_in = nc.declare_dram_parameter("xt", [DIM, SHARD], bf16, isOutput=False)
    c_in = nc.declare_dram_parameter("consts", [DIM, _C_COLS], bf16, isOutput=False)
    out_ext = nc.declare_dram_parameter("out", [DIM, SHARD], f32, isOutput=True)

    with TileContext(nc) as tc:
        with (
            tc.tile_pool(name="io", bufs=1) as iopool,
            tc.tile_pool(name="mid", bufs=1) as midpool,
            tc.tile_pool(name="mm_ps", bufs=3, space="PSUM") as mmpool,
            tc.tile_pool(name="wu_ps", bufs=1, space="PSUM") as wupool,
        ):
            # small consts first on sync so matmul weights land early
            ct = iopool.tile([DIM, _C_COLS], bf16, tag="consts")
            nc.sync.dma_start(ct[:, :], c_in[:, :])
            wmi_t = ct[:, _C_WMI : _C_WMI + DIM]
            wmi2_t = ct[:, _C_WMI2 : _C_WMI2 + DIM]
            mlh_t = ct[:, _C_MLH : _C_MLH + DIM]
            sb_t = ct[:, _C_SB : _C_SB + 2].bitcast(f32)
            c1_t = ct[:, _C_C1 : _C_C1 + 2].bitcast(f32)

            # input chunks: sync + gpsimd issue in parallel
            xT = iopool.tile([DIM, SHARD], bf16, tag="xT")
            in_eng = [nc.sync, nc.gpsimd, nc.sync, nc.gpsimd, nc.gpsimd]
            for c, (beg, sz) in enumerate(_CHUNKS):
                cs = slice(beg, beg + sz)
                in_eng[c].dma_start(xT[:, cs], xt_in[:, cs])

            # PE p-state warmup: stream the consts tile through the array
            wu = wupool.tile([DIM, 256], f32, tag="wu")
            for _ in range(_N_WARMUP):
                nc.tensor.matmul(
                    wu[:], lhsT=wmi_t, rhs=ct[:, 0:256], start=True, stop=True,
                )

            axs, sgs, lgs, pss = [], [], [], []
            for c, (beg, sz) in enumerate(_CHUNKS):
                cs = slice(beg, beg + sz)
                ax = midpool.tile([DIM, sz], bf16, tag=f"ax{c}")
                if _AX_ENG[c] == "dve":
                    # ax = |x| via sign-bit clear. (The reference's eps clamp
                    # is dropped on-device: the host verifies no |x| < eps
                    # and falls back otherwise.)
                    nc.vector.tensor_scalar(
                        ax[:].bitcast(u16), xT[:, cs].bitcast(u16),
                        0x7FFF, None, Alu.bitwise_and,
                    )
                else:
                    # ax = x^2 on the Pool engine; Ln(x^2) = 2*Ln|x| and the
                    # 1/2 is folded into this chunk's matmul weights.
                    nc.gpsimd.tensor_tensor(ax[:], xT[:, cs], xT[:, cs], Alu.mult)
                axs.append(ax)
            if use_sg:
                for c, (beg, sz) in enumerate(_CHUNKS):
                    cs = slice(beg, beg + sz)
                    sg = midpool.tile([DIM, sz], bf16, tag=f"sg{c}")
                    nc.vector.tensor_scalar(
                        sg[:].bitcast(u16), xT[:, cs].bitcast(u16),
                        0x8000, 0x3F80, Alu.bitwise_and, Alu.bitwise_or,
                    )
                    sgs.append(sg)
            for c, (beg, sz) in enumerate(_CHUNKS):
                lg = midpool.tile([DIM, sz], bf16, tag=f"lg{c}")
                nc.scalar.activation(lg[:], axs[c][:], Act.Ln)
                lgs.append(lg)
            for c, (beg, sz) in enumerate(_CHUNKS):
                w_t = wmi_t if _AX_ENG[c] == "dve" else wmi2_t
                ps = mmpool.tile([DIM, sz], f32, tag="mm")
                for k in range(sz // 512):
                    ks = slice(k * 512, (k + 1) * 512)
                    nc.tensor.matmul(
                        ps[:, ks], lhsT=w_t, rhs=lgs[c][:, ks],
                        start=True, stop=not use_sg,
                    )
                if use_sg:
                    for k in range(sz // 512):
                        ks = slice(k * 512, (k + 1) * 512)
                        nc.tensor.matmul(
                            ps[:, ks], lhsT=mlh_t, rhs=sgs[c][:, ks],
                            start=False, stop=True,
                        )
                pss.append(ps)
            for c, (beg, sz) in enumerate(_CHUNKS):
                cs = slice(beg, beg + sz)
                # out = x * (c1 + sb*(ps + 0.5*ps^2)) in one fused DVE pass
                oT = midpool.tile([DIM, sz], f32, tag=f"oT{c}")
                nc.vector._custom_dve(
                    dve_op, out=oT[:], in0=pss[c][:], in1=xT[:, cs],
                    s0=sb_t, s1=c1_t, imm2=0.5,
                )
                nc.sync.dma_start(out_ext[:, cs], oT[:])

    nc.finalize()
    return nc


def _get_program(use_sg=False):
    if use_sg not in _PROGRAMS:
        _PROGRAMS[use_sg] = _build_program(use_sg)
    return _PROGRAMS[use_sg]


def _host_inputs(x, W_m, M_m, G):
    """Host-side parameter precompute shared by kernel() and test harness.

    Returns (in_maps, aux); aux["mode"] is "fast" (fluctuation dropped),
    "sg" (exact sign matmul), or "host" (full CPU fallback)."""
    dim = DIM
    eye = np.eye(dim, dtype=np.float32)
    wm = eye + (1.0 - eye) * np.tanh(W_m) * (1.0 / (1.0 + np.exp(-M_m)))
    wm = wm.astype(np.float32)
    a = np.abs(wm)
    one_m_2a = 1.0 - 2.0 * a
    with np.errstate(divide="ignore"):
        L = np.log(np.abs(one_m_2a)).astype(np.float32)
    np.fill_diagonal(L, 0.0)
    g = np.tanh(G).astype(np.float32)

    # --- device-path validity checks (cheap, O(N d + d^2)) ---------------
    off = one_m_2a.copy()
    np.fill_diagonal(off, 1.0)
    sign_ok = bool((off > 0.0).all())

    xbf = x.astype(BF16)
    absx = np.abs(xbf.astype(np.float32))
    eps_ok = bool((absx >= EPS).all())

    max_absx = float(absx.max()) if absx.size else 1.0
    max_lg = np.log(max(max_absx, EPS))
    maxabs_lg = max(abs(np.log(EPS)), abs(max_lg))
    a_off = a - np.diag(np.diag(a))
    s_off = float(a_off.sum(axis=0).max())
    omega_ok = bool(max_lg + maxabs_lg * s_off < OMEGA - 0.25)
    # Taylor validity: |exp argument| bound small enough for 2nd order
    fl_bound = 0.5 * float(np.abs(L).sum(axis=0).max())
    taylor_ok = bool(maxabs_lg * s_off + fl_bound < 0.25)
    # sign-fluctuation term droppable when its relative effect is tiny
    drop_ok = bool(np.expm1(fl_bound) < 5e-3)

    if sign_ok and eps_ok and omega_ok and taylor_ok:
        mode = "fast" if drop_ok else "sg"
    else:
        mode = "host"

    # --- packed constants -------------------------------------------------
    wmi = (wm - eye).astype(BF16)
    wmi2 = (0.5 * (wm - eye)).astype(BF16)
    mlh = (-0.5 * L).astype(BF16)
    colsum = 0.5 * L.sum(axis=0, dtype=np.float64)
    sb = (g.astype(np.float64) * np.exp(colsum)).astype(np.float32)
    c1 = (1.0 + sb).astype(np.float32)
    sb_u16 = sb.view(np.uint16).reshape(dim, 2)
    c1_u16 = c1.view(np.uint16).reshape(dim, 2)

    consts = np.zeros((dim, _C_COLS), dtype=np.uint16)
    consts[:, _C_WMI : _C_WMI + dim] = wmi.view(np.uint16)
    consts[:, _C_WMI2 : _C_WMI2 + dim] = wmi2.view(np.uint16)
    consts[:, _C_MLH : _C_MLH + dim] = mlh.view(np.uint16)
    consts[:, _C_SB] = sb_u16[:, 0]
    consts[:, _C_SB + 1] = sb_u16[:, 1]
    consts[:, _C_C1] = c1_u16[:, 0]
    consts[:, _C_C1 + 1] = c1_u16[:, 1]
    consts_bf = consts.view(BF16)

    in_maps = []
    for cid in range(N_CORES):
        rows = slice(cid * SHARD, (cid + 1) * SHARD)
        in_maps.append(
            {
                "xt": np.ascontiguousarray(xbf[rows].T),
                "consts": consts_bf,
            }
        )

    aux = {"wm": wm, "a": a, "one_m_2a": one_m_2a, "g": g, "mode": mode}
    return in_maps, aux


def kernel(x, W_m, M_m, G):
    from concourse.bass_utils import run_bass_kernel_spmd

    x = np.asarray(x, dtype=np.float32)
    W_m = np.asarray(W_m, dtype=np.float32)
    M_m = np.asarray(M_m, dtype=np.float32)
    G = np.asarray(G, dtype=np.float32)

    in_maps, aux = _host_inputs(x, W_m, M_m, G)

    if aux["mode"] == "host":
        # General-case fixup (never taken for the reference data):
        # compute the output exactly on the host.
        wm, a, one_m_2a, g = aux["wm"], aux["a"], aux["one_m_2a"], aux["g"]
        lg_h = np.log(np.maximum(np.abs(x), EPS))
        ls = lg_h @ wm
        mul = np.exp(np.minimum(ls, OMEGA))
        msv = np.ones_like(x)
        for i in range(DIM):
            f = np.where(
                x[:, i : i + 1] > 0,
                1.0,
                np.where(x[:, i : i + 1] < 0, one_m_2a[i], 1.0 - a[i]),
            )
            msv *= f
        return (x + mul * msv * g).astype(np.float32)

    nc = _get_program(use_sg=(aux["mode"] == "sg"))
    res = run_bass_kernel_spmd(nc, in_maps, core_ids=list(range(N_CORES)))
    out = np.empty((N_TOTAL, DIM), dtype=np.float32)
    for cid, r in enumerate(res.results):
        rows = slice(cid * SHARD, (cid + 1) * SHARD)
        out[rows] = r["out"].T
    return out


# revision 16
# speedup vs baseline: 1.3444x; 1.0457x over previous
"""NALU layer kernel for Trainium2, data-parallel across 8 NeuronCores.

Reference computation (dim=128, N=32768, eps=1e-7, omega=20):
    wm  = I + (1-I) * tanh(W_m) * sigmoid(M_m)             [d, d]
    ls  = log(max(|x|, eps)) @ wm                          [N, d]
    mul = exp(min(ls, omega))
    msm = sign(x)[:, :, None] * |wm| + (1 - |wm|)          [N, d, d]
    msv = prod(msm, axis=1)                                [N, d]
    out = x + mul * msv * tanh(G)

Restructure (no [N,d,d] product, no on-device transposes, x factored out,
exp replaced by a 2nd-order Taylor of its provably-tiny argument):
    With sigma = sign(x) in {-1,+1} (x==0 / |x|<eps host-checked), and
    L[i,j] = log|1-2|wm[i,j]||  (L[j,j]=0 since |wm[j,j]|=1),
        msv[n,j] = sigma[n,j] * exp( 0.5*colsum_L[j] - sigma[n,:] @ (L[:,j]/2) )
    (off-diagonal (1-2|wm|) > 0 host-verified; diagonal carries the sign).
    Since exp(lg[n,j]) = |x[n,j]| (no |x|<eps, host-verified):
        out[n,j] = x * (1 + sb_j * exp(eps_mm[n,j] + fl[n,j]))
        eps_mm   = lg @ (wm - I)
        fl       = -sigma @ (L/2)        (zero-mean sign fluctuation)
        sb_j     = tanh(G_j) * exp(0.5*colsum_L[j])   (exactly 0 when G==0)
    |fl| <= 0.5*max_colsum|L| (~3e-3 for these weights): when the
    host-computed bound keeps its effect under 0.5% relative it is dropped
    (comparable to the bf16 input rounding of 0.4%); otherwise an alternate
    program that computes it exactly (one more matmul accumulating
    sigma @ (-L/2)) is used.
    |eps_mm| <= max|lg| * max_colsum_offdiag|wm| (~0.05, host-verified
    < 0.25) so exp(z) = 1 + z + z^2/2 to <= 3e-4 relative, and the whole
    tail fuses into ONE custom DVE pass:
        out = x * (c1_j + sb_j * (z + 0.5*z^2)),   c1_j = 1 + sb_j
    The omega clamp is host-verified to never bind (cheap upper bound).

Layout: everything feature-major. The HOST ships x^T as bf16 [d, shard]
(features on partitions) so per-partition DMA lines are large and
contiguous; the device writes the f32 output feature-major as well and
the host transposes it back. Per-feature constants (sb, c1) become
per-partition DVE scalars. Device pipeline per column-chunk:
    DVE or Pool : ax = |x| (DVE bit op) or x^2 (Pool tensor_tensor;
                  Ln(x^2) = 2 Ln|x|, the 1/2 folds into that chunk's weights)
    ACT         : lg = Ln(ax)
    PE          : ps = wmI^T.lg      (accumulating matmuls per 512 cols)
    DVE         : oT = x * (c1 + sb*(ps + ps^2/2))   (one fused custom op)
Input DMAs issue from sync + gpsimd in parallel, stores from sync in chunk
order; a few dummy matmuls on the consts tile warm the PE out of its low
p-state while the input streams in.
With the reference G == 0: sb == 0, c1 == 1 exactly, so out == bf16(x) and
the only error vs the f32 reference is the bf16 rounding of x (<= 2^-8).
"""

import sys

for _p in ("/opt/trn_rl_repo",):
    if _p not in sys.path:
        sys.path.insert(0, _p)

import numpy as np
import ml_dtypes

DIM = 128
N_TOTAL = 32768
N_CORES = 8
SHARD = N_TOTAL // N_CORES          # 4096 rows per core
EPS = 1e-07
OMEGA = 20.0

BF16 = ml_dtypes.bfloat16

# column-chunks of the [DIM, SHARD] feature-major tile: small first chunk to
# prime the pipe, small last chunk to shorten the store tail
_CHUNKS = [(0, 512), (512, 1024), (1536, 1024), (2560, 1024), (3584, 512)]
# which engine computes the Ln input per chunk: "dve" -> |x|, "pool" -> x^2
_AX_ENG = ["dve", "pool", "dve", "pool", "pool"]

# consts tile columns: wmI | wmI/2 | -L/2 | sb(f32 2 cols) | c1(f32) | pad
_C_WMI = 0
_C_WMI2 = DIM
_C_MLH = 2 * DIM
_C_SB = 3 * DIM
_C_C1 = 3 * DIM + 2
_C_COLS = 3 * DIM + 8

_N_WARMUP = 6                       # dummy 256-col matmuls before real work

_PROGRAMS = {}
_DVE_OP = None


def _patch_act_tables(bacc_mod):
    """Make Ln/Exp resolve only to the combined natural_log_exp set, so the
    table-load pass emits a single ACT_TABLE_LOAD for the Ln chain."""
    from concourse import mybir

    orig = bacc_mod.get_activation_tables
    if getattr(orig, "_nalu_patched", False):
        return

    def patched(module_arch):
        tabs = orig(module_arch)
        both = {mybir.ActivationFunctionType.Ln, mybir.ActivationFunctionType.Exp}
        for name, fns in tabs.items():
            if name != "natural_log_exp_and_others":
                fns -= both
        return tabs

    patched._nalu_patched = True
    bacc_mod.get_activation_tables = patched


def _get_dve_op():
    """Register (once) the fused NALU tail as a custom DVE op:
        out = Src1 * (C1 + C0 * (Src0 + Src0^2 * imm2))
    with Src0 = eps_mm (psum f32), Src1 = x (bf16), C0 = sb[j], C1 = c1[j]
    per-partition f32 scalars, imm2 = 0.5."""
    global _DVE_OP
    if _DVE_OP is not None:
        return _DVE_OP
    from concourse import dve_ops
    from concourse.dve_spec import Spec, Src0, Src1, C0, C1, C2, sq, lower

    name = "NALU_V_FUSED_ANT"
    for op in dve_ops.OPS:
        if op.name == name:
            _DVE_OP = op
            return op
    spec = Spec(body=Src1 * (C1 + C0 * (Src0 + sq(Src0) * C2)))
    row = max(dve_ops._SUB_OPCODE_FOR_NAME.values()) + 1
    dve_ops._SUB_OPCODE_FOR_NAME[name] = row
    shas = {}
    for ver in ("v3", "v4"):
        shas[ver] = dve_ops.DveOpSpec(
            name=name, opcode=row, uops=lower(spec, ver=ver),
            rd1_en=dve_ops.has_src1(spec),
        ).sha(ver)
    op = dve_ops.DveOp(name, spec, subdim=False, uops_sha=shas)
    dve_ops.OPS.append(op)
    dve_ops.CUSTOM_DVE_SPECS[name] = spec
    _DVE_OP = op
    return op


def _build_program(use_sg):
    from concourse import bacc, mybir
    from concourse.tile import TileContext

    _patch_act_tables(bacc)
    dve_op = _get_dve_op()

    f32 = mybir.dt.float32
    bf16 = mybir.dt.bfloat16
    u16 = mybir.dt.uint16
    Alu = mybir.AluOpType
    Act = mybir.ActivationFunctionType

    nc = bacc.Bacc("TRN2", target_bir_lowering=False)

    xt_in = nc.declare_dram_parameter("xt", [DIM, SHARD], bf16, isOutput=False)
    c_in = nc.declare_dram_parameter("consts", [DIM, _C_COLS], bf16, isOutput=False)
    out_ext = nc.declare_dram_parameter("out", [DIM, SHARD], f32, isOutput=True)

    with TileContext(nc) as tc:
        with (
            tc.tile_pool(name="io", bufs=1) as iopool,
            tc.tile_pool(name="mid", bufs=1) as midpool,
            tc.tile_pool(name="mm_ps", bufs=3, space="PSUM") as mmpool,
            tc.tile_pool(name="wu_ps", bufs=1, space="PSUM") as wupool,
        ):
            # small consts first on sync so matmul weights land early
            ct = iopool.tile([DIM, _C_COLS], bf16, tag="consts")
            nc.sync.dma_start(ct[:, :], c_in[:, :])
            wmi_t = ct[:, _C_WMI : _C_WMI + DIM]
            wmi2_t = ct[:, _C_WMI2 : _C_WMI2 + DIM]
            mlh_t = ct[:, _C_MLH : _C_MLH + DIM]
            sb_t = ct[:, _C_SB : _C_SB + 2].bitcast(f32)
            c1_t = ct[:, _C_C1 : _C_C1 + 2].bitcast(f32)

            # input chunks all issue from sync in chunk order: serialized
            # issues stagger the transfers so chunk 0 owns the DMA bus first
            # and lands ~2us before the tail chunks (parallel issues from
            # several engines made every chunk finish together, late).
            xT = iopool.tile([DIM, SHARD], bf16, tag="xT")
            for c, (beg, sz) in enumerate(_CHUNKS):
                cs = slice(beg, beg + sz)
                nc.sync.dma_start(xT[:, cs], xt_in[:, cs])

            # PE p-state warmup: stream the consts tile through the array
            wu = wupool.tile([DIM, 256], f32, tag="wu")
            for _ in range(_N_WARMUP):
                nc.tensor.matmul(
                    wu[:], lhsT=wmi_t, rhs=ct[:, 0:256], start=True, stop=True,
                )

            axs = [None] * len(_CHUNKS)
            sgs, lgs, pss = [], [], []
            # emit DVE |x| chunks before Pool x^2 chunks so the DVE queue is
            # [ax..., fused...] with every ax ready before the first fused
            for c, (beg, sz) in sorted(
                enumerate(_CHUNKS), key=lambda t: _AX_ENG[t[0]] != "dve"
            ):
                cs = slice(beg, beg + sz)
                ax = midpool.tile([DIM, sz], bf16, tag=f"ax{c}")
                if _AX_ENG[c] == "dve":
                    # ax = |x| via sign-bit clear. (The reference's eps clamp
                    # is dropped on-device: the host verifies no |x| < eps
                    # and falls back otherwise.)
                    nc.vector.tensor_scalar(
                        ax[:].bitcast(u16), xT[:, cs].bitcast(u16),
                        0x7FFF, None, Alu.bitwise_and,
                    )
                else:
                    # ax = x^2 on the Pool engine; Ln(x^2) = 2*Ln|x| and the
                    # 1/2 is folded into this chunk's matmul weights.
                    nc.gpsimd.tensor_tensor(ax[:], xT[:, cs], xT[:, cs], Alu.mult)
                axs[c] = ax
            if use_sg:
                for c, (beg, sz) in enumerate(_CHUNKS):
                    cs = slice(beg, beg + sz)
                    sg = midpool.tile([DIM, sz], bf16, tag=f"sg{c}")
                    nc.vector.tensor_scalar(
                        sg[:].bitcast(u16), xT[:, cs].bitcast(u16),
                        0x8000, 0x3F80, Alu.bitwise_and, Alu.bitwise_or,
                    )
                    sgs.append(sg)
            for c, (beg, sz) in enumerate(_CHUNKS):
                lg = midpool.tile([DIM, sz], bf16, tag=f"lg{c}")
                nc.scalar.activation(lg[:], axs[c][:], Act.Ln)
                lgs.append(lg)
            for c, (beg, sz) in enumerate(_CHUNKS):
                w_t = wmi_t if _AX_ENG[c] == "dve" else wmi2_t
                ps = mmpool.tile([DIM, sz], f32, tag="mm")
                for k in range(sz // 512):
                    ks = slice(k * 512, (k + 1) * 512)
                    nc.tensor.matmul(
                        ps[:, ks], lhsT=w_t, rhs=lgs[c][:, ks],
                        start=True, stop=not use_sg,
                    )
                if use_sg:
                    for k in range(sz // 512):
                        ks = slice(k * 512, (k + 1) * 512)
                        nc.tensor.matmul(
                            ps[:, ks], lhsT=mlh_t, rhs=sgs[c][:, ks],
                            start=False, stop=True,
                        )
                pss.append(ps)
            for c, (beg, sz) in enumerate(_CHUNKS):
                cs = slice(beg, beg + sz)
                # out = x * (c1 + sb*(ps + 0.5*ps^2)) in one fused DVE pass
                oT = midpool.tile([DIM, sz], f32, tag=f"oT{c}")
                nc.vector._custom_dve(
                    dve_op, out=oT[:], in0=pss[c][:], in1=xT[:, cs],
                    s0=sb_t, s1=c1_t, imm2=0.5,
                )
                nc.sync.dma_start(out_ext[:, cs], oT[:])

    nc.finalize()
    return nc


def _get_program(use_sg=False):
    if use_sg not in _PROGRAMS:
        _PROGRAMS[use_sg] = _build_program(use_sg)
    return _PROGRAMS[use_sg]


def _host_inputs(x, W_m, M_m, G):
    """Host-side parameter precompute shared by kernel() and test harness.

    Returns (in_maps, aux); aux["mode"] is "fast" (fluctuation dropped),
    "sg" (exact sign matmul), or "host" (full CPU fallback)."""
    dim = DIM
    eye = np.eye(dim, dtype=np.float32)
    wm = eye + (1.0 - eye) * np.tanh(W_m) * (1.0 / (1.0 + np.exp(-M_m)))
    wm = wm.astype(np.float32)
    a = np.abs(wm)
    one_m_2a = 1.0 - 2.0 * a
    with np.errstate(divide="ignore"):
        L = np.log(np.abs(one_m_2a)).astype(np.float32)
    np.fill_diagonal(L, 0.0)
    g = np.tanh(G).astype(np.float32)

    # --- device-path validity checks (cheap, O(N d + d^2)) ---------------
    off = one_m_2a.copy()
    np.fill_diagonal(off, 1.0)
    sign_ok = bool((off > 0.0).all())

    xbf = x.astype(BF16)
    absx = np.abs(xbf.astype(np.float32))
    eps_ok = bool((absx >= EPS).all())

    max_absx = float(absx.max()) if absx.size else 1.0
    max_lg = np.log(max(max_absx, EPS))
    maxabs_lg = max(abs(np.log(EPS)), abs(max_lg))
    a_off = a - np.diag(np.diag(a))
    s_off = float(a_off.sum(axis=0).max())
    omega_ok = bool(max_lg + maxabs_lg * s_off < OMEGA - 0.25)
    # Taylor validity: |exp argument| bound small enough for 2nd order
    fl_bound = 0.5 * float(np.abs(L).sum(axis=0).max())
    taylor_ok = bool(maxabs_lg * s_off + fl_bound < 0.25)
    # sign-fluctuation term droppable when its relative effect is tiny
    drop_ok = bool(np.expm1(fl_bound) < 5e-3)

    if sign_ok and eps_ok and omega_ok and taylor_ok:
        mode = "fast" if drop_ok else "sg"
    else:
        mode = "host"

    # --- packed constants -------------------------------------------------
    wmi = (wm - eye).astype(BF16)
    wmi2 = (0.5 * (wm - eye)).astype(BF16)
    mlh = (-0.5 * L).astype(BF16)
    colsum = 0.5 * L.sum(axis=0, dtype=np.float64)
    sb = (g.astype(np.float64) * np.exp(colsum)).astype(np.float32)
    c1 = (1.0 + sb).astype(np.float32)
    sb_u16 = sb.view(np.uint16).reshape(dim, 2)
    c1_u16 = c1.view(np.uint16).reshape(dim, 2)

    consts = np.zeros((dim, _C_COLS), dtype=np.uint16)
    consts[:, _C_WMI : _C_WMI + dim] = wmi.view(np.uint16)
    consts[:, _C_WMI2 : _C_WMI2 + dim] = wmi2.view(np.uint16)
    consts[:, _C_MLH : _C_MLH + dim] = mlh.view(np.uint16)
    consts[:, _C_SB] = sb_u16[:, 0]
    consts[:, _C_SB + 1] = sb_u16[:, 1]
    consts[:, _C_C1] = c1_u16[:, 0]
    consts[:, _C_C1 + 1] = c1_u16[:, 1]
    consts_bf = consts.view(BF16)

    in_maps = []
    for cid in range(N_CORES):
        rows = slice(cid * SHARD, (cid + 1) * SHARD)
        in_maps.append(
            {
                "xt": np.ascontiguousarray(xbf[rows].T),
                "consts": consts_bf,
            }
        )

    aux = {"wm": wm, "a": a, "one_m_2a": one_m_2a, "g": g, "mode": mode}
    return in_maps, aux


def kernel(x, W_m, M_m, G):
    from concourse.bass_utils import run_bass_kernel_spmd

    x = np.asarray(x, dtype=np.float32)
    W_m = np.asarray(W_m, dtype=np.float32)
    M_m = np.asarray(M_m, dtype=np.float32)
    G = np.asarray(G, dtype=np.float32)

    in_maps, aux = _host_inputs(x, W_m, M_m, G)

    if aux["mode"] == "host":
        # General-case fixup (never taken for the reference data):
        # compute the output exactly on the host.
        wm, a, one_m_2a, g = aux["wm"], aux["a"], aux["one_m_2a"], aux["g"]
        lg_h = np.log(np.maximum(np.abs(x), EPS))
        ls = lg_h @ wm
        mul = np.exp(np.minimum(ls, OMEGA))
        msv = np.ones_like(x)
        for i in range(DIM):
            f = np.where(
                x[:, i : i + 1] > 0,
                1.0,
                np.where(x[:, i : i + 1] < 0, one_m_2a[i], 1.0 - a[i]),
            )
            msv *= f
        return (x + mul * msv * g).astype(np.float32)

    nc = _get_program(use_sg=(aux["mode"] == "sg"))
    res = run_bass_kernel_spmd(nc, in_maps, core_ids=list(range(N_CORES)))
    out = np.empty((N_TOTAL, DIM), dtype=np.float32)
    for cid, r in enumerate(res.results):
        rows = slice(cid * SHARD, (cid + 1) * SHARD)
        out[rows] = r["out"].T
    return out


# revision 21
# speedup vs baseline: 1.3609x; 1.0123x over previous
"""NALU layer kernel for Trainium2, data-parallel across 8 NeuronCores.

Reference computation (dim=128, N=32768, eps=1e-7, omega=20):
    wm  = I + (1-I) * tanh(W_m) * sigmoid(M_m)             [d, d]
    ls  = log(max(|x|, eps)) @ wm                          [N, d]
    mul = exp(min(ls, omega))
    msm = sign(x)[:, :, None] * |wm| + (1 - |wm|)          [N, d, d]
    msv = prod(msm, axis=1)                                [N, d]
    out = x + mul * msv * tanh(G)

Restructure (no [N,d,d] product, no on-device transposes, x factored out,
exp replaced by a 2nd-order Taylor of its provably-tiny argument):
    With sigma = sign(x) in {-1,+1} (x==0 / |x|<eps host-checked), and
    L[i,j] = log|1-2|wm[i,j]||  (L[j,j]=0 since |wm[j,j]|=1),
        msv[n,j] = sigma[n,j] * exp( 0.5*colsum_L[j] - sigma[n,:] @ (L[:,j]/2) )
    (off-diagonal (1-2|wm|) > 0 host-verified; diagonal carries the sign).
    Since exp(lg[n,j]) = |x[n,j]| (no |x|<eps, host-verified):
        out[n,j] = x * (1 + sb_j * exp(eps_mm[n,j] + fl[n,j]))
        eps_mm   = lg @ (wm - I)
        fl       = -sigma @ (L/2)        (zero-mean sign fluctuation)
        sb_j     = tanh(G_j) * exp(0.5*colsum_L[j])   (exactly 0 when G==0)
    |fl| <= 0.5*max_colsum|L| (~3e-3 for these weights): when the
    host-computed bound keeps its effect under 0.5% relative it is dropped
    (comparable to the bf16 input rounding of 0.4%); otherwise an alternate
    program that computes it exactly (one more matmul accumulating
    sigma @ (-L/2)) is used.
    |eps_mm| <= max|lg| * max_colsum_offdiag|wm| (~0.05, host-verified
    < 0.25) so exp(z) = 1 + z + z^2/2 to <= 3e-4 relative, and the whole
    tail fuses into ONE custom DVE pass:
        out = x * (c1_j + sb_j * (z + 0.5*z^2)),   c1_j = 1 + sb_j
    The omega clamp is host-verified to never bind (cheap upper bound).

Layout: everything feature-major. The HOST ships x^T as bf16 [d, shard]
(features on partitions) so per-partition DMA lines are large and
contiguous; the device writes the f32 output feature-major as well and
the host transposes it back. Per-feature constants (sb, c1) become
per-partition DVE scalars. Device pipeline per column-chunk:
    DVE or Pool : ax = |x| (DVE bit op) or x^2 (Pool tensor_tensor;
                  Ln(x^2) = 2 Ln|x|, the 1/2 folds into that chunk's weights)
    ACT         : lg = Ln(ax)
    PE          : ps = wmI^T.lg      (accumulating matmuls per 512 cols)
    DVE         : oT = x * (c1 + sb*(ps + ps^2/2))   (one fused custom op)
Input DMAs issue from sync + gpsimd in parallel, stores from sync in chunk
order; a few dummy matmuls on the consts tile warm the PE out of its low
p-state while the input streams in.
With the reference G == 0: sb == 0, c1 == 1 exactly, so out == bf16(x) and
the only error vs the f32 reference is the bf16 rounding of x (<= 2^-8).
"""

import sys

for _p in ("/opt/trn_rl_repo",):
    if _p not in sys.path:
        sys.path.insert(0, _p)

import numpy as np
import ml_dtypes

DIM = 128
N_TOTAL = 32768
N_CORES = 8
SHARD = N_TOTAL // N_CORES          # 4096 rows per core
EPS = 1e-07
OMEGA = 20.0

BF16 = ml_dtypes.bfloat16

# column-chunks of the [DIM, SHARD] feature-major tile: small first chunk to
# prime the pipe, small last chunk to shorten the store tail
_CHUNKS = [(0, 512), (512, 1024), (1536, 1024), (2560, 1024), (3584, 512)]
# every chunk's Ln input is x^2 computed on DVE (bf16 tensor_tensor runs at
# 2 elem/cycle there vs Pool's 0.42-efficiency software loop); Ln(x^2) =
# 2*Ln|x| and the 1/2 folds into the matmul weights

# consts tile columns: wmI | wmI/2 | -L/2 | sb(f32 2 cols) | c1(f32) | pad
_C_WMI = 0
_C_WMI2 = DIM
_C_MLH = 2 * DIM
_C_SB = 3 * DIM
_C_C1 = 3 * DIM + 2
_C_COLS = 3 * DIM + 8

_N_WARMUP = 4                       # dummy 256-col matmuls before real work

_PROGRAMS = {}
_DVE_OP = None


def _patch_act_tables(bacc_mod):
    """Make Ln/Exp resolve only to the combined natural_log_exp set, so the
    table-load pass emits a single ACT_TABLE_LOAD for the Ln chain."""
    from concourse import mybir

    orig = bacc_mod.get_activation_tables
    if getattr(orig, "_nalu_patched", False):
        return

    def patched(module_arch):
        tabs = orig(module_arch)
        both = {mybir.ActivationFunctionType.Ln, mybir.ActivationFunctionType.Exp}
        for name, fns in tabs.items():
            if name != "natural_log_exp_and_others":
                fns -= both
        return tabs

    patched._nalu_patched = True
    bacc_mod.get_activation_tables = patched


def _get_dve_op():
    """Register (once) the fused NALU tail as a custom DVE op:
        out = Src1 * (C1 + C0 * (Src0 + Src0^2 * imm2))
    with Src0 = eps_mm (psum f32), Src1 = x (bf16), C0 = sb[j], C1 = c1[j]
    per-partition f32 scalars, imm2 = 0.5."""
    global _DVE_OP
    if _DVE_OP is not None:
        return _DVE_OP
    from concourse import dve_ops
    from concourse.dve_spec import Spec, Src0, Src1, C0, C1, C2, sq, lower

    name = "NALU_V_FUSED_ANT"
    for op in dve_ops.OPS:
        if op.name == name:
            _DVE_OP = op
            return op
    spec = Spec(body=Src1 * (C1 + C0 * (Src0 + sq(Src0) * C2)))
    row = max(dve_ops._SUB_OPCODE_FOR_NAME.values()) + 1
    dve_ops._SUB_OPCODE_FOR_NAME[name] = row
    shas = {}
    for ver in ("v3", "v4"):
        shas[ver] = dve_ops.DveOpSpec(
            name=name, opcode=row, uops=lower(spec, ver=ver),
            rd1_en=dve_ops.has_src1(spec),
        ).sha(ver)
    op = dve_ops.DveOp(name, spec, subdim=False, uops_sha=shas)
    dve_ops.OPS.append(op)
    dve_ops.CUSTOM_DVE_SPECS[name] = spec
    _DVE_OP = op
    return op


def _build_program(use_sg):
    from concourse import bacc, mybir
    from concourse.tile import TileContext

    _patch_act_tables(bacc)
    dve_op = _get_dve_op()

    f32 = mybir.dt.float32
    bf16 = mybir.dt.bfloat16
    u16 = mybir.dt.uint16
    Alu = mybir.AluOpType
    Act = mybir.ActivationFunctionType

    nc = bacc.Bacc("TRN2", target_bir_lowering=False)

    xt_in = nc.declare_dram_parameter("xt", [DIM, SHARD], bf16, isOutput=False)
    c_in = nc.declare_dram_parameter("consts", [DIM, _C_COLS], bf16, isOutput=False)
    out_ext = nc.declare_dram_parameter("out", [DIM, SHARD], f32, isOutput=True)

    with TileContext(nc) as tc:
        with (
            tc.tile_pool(name="io", bufs=1) as iopool,
            tc.tile_pool(name="mid", bufs=1) as midpool,
            tc.tile_pool(name="mm_ps", bufs=3, space="PSUM") as mmpool,
            tc.tile_pool(name="wu_ps", bufs=1, space="PSUM") as wupool,
        ):
            # consts issue from scalar (ahead of its act-table load); the
            # first x chunk owns the sync queue and the DMA bus immediately
            ct = iopool.tile([DIM, _C_COLS], bf16, tag="consts")
            nc.scalar.dma_start(ct[:, :], c_in[:, :])
            wmi_t = ct[:, _C_WMI : _C_WMI + DIM]
            wmi2_t = ct[:, _C_WMI2 : _C_WMI2 + DIM]
            mlh_t = ct[:, _C_MLH : _C_MLH + DIM]
            sb_t = ct[:, _C_SB : _C_SB + 2].bitcast(f32)
            c1_t = ct[:, _C_C1 : _C_C1 + 2].bitcast(f32)

            # input chunks all issue from sync in chunk order: serialized
            # issues stagger the transfers so chunk 0 owns the DMA bus first
            # and lands ~2us before the tail chunks (parallel issues from
            # several engines made every chunk finish together, late).
            xT = iopool.tile([DIM, SHARD], bf16, tag="xT")
            for c, (beg, sz) in enumerate(_CHUNKS):
                cs = slice(beg, beg + sz)
                nc.sync.dma_start(xT[:, cs], xt_in[:, cs])

            # PE p-state warmup: stream the consts tile through the array
            wu = wupool.tile([DIM, 256], f32, tag="wu")
            for _ in range(_N_WARMUP):
                nc.tensor.matmul(
                    wu[:], lhsT=wmi_t, rhs=ct[:, 0:256], start=True, stop=True,
                )

            axs = [None] * len(_CHUNKS)
            sgs, lgs, pss = [], [], []
            # x^2 for the first three chunks up front; chunks 3/4 are
            # emitted interleaved with the fused ops below so the DVE queue
            # never holds a ready fused op behind a not-yet-ready x^2
            def emit_ax(c):
                beg, sz = _CHUNKS[c]
                cs = slice(beg, beg + sz)
                ax = midpool.tile([DIM, sz], bf16, tag=f"ax{c}")
                nc.vector.tensor_tensor(ax[:], xT[:, cs], xT[:, cs], Alu.mult)
                axs[c] = ax

            def emit_sg(c):
                beg, sz = _CHUNKS[c]
                cs = slice(beg, beg + sz)
                sg = midpool.tile([DIM, sz], bf16, tag=f"sg{c}")
                nc.vector.tensor_scalar(
                    sg[:].bitcast(u16), xT[:, cs].bitcast(u16),
                    0x8000, 0x3F80, Alu.bitwise_and, Alu.bitwise_or,
                )
                sgs[c] = sg

            def emit_ln_mm(c):
                beg, sz = _CHUNKS[c]
                lg = midpool.tile([DIM, sz], bf16, tag=f"lg{c}")
                nc.scalar.activation(lg[:], axs[c][:], Act.Ln)
                ps = mmpool.tile([DIM, sz], f32, tag="mm")
                for k in range(sz // 512):
                    ks = slice(k * 512, (k + 1) * 512)
                    nc.tensor.matmul(
                        ps[:, ks], lhsT=wmi2_t, rhs=lg[:, ks],
                        start=True, stop=not use_sg,
                    )
                if use_sg:
                    for k in range(sz // 512):
                        ks = slice(k * 512, (k + 1) * 512)
                        nc.tensor.matmul(
                            ps[:, ks], lhsT=mlh_t, rhs=sgs[c][:, ks],
                            start=False, stop=True,
                        )
                pss[c] = ps

            def emit_fused(c):
                beg, sz = _CHUNKS[c]
                cs = slice(beg, beg + sz)
                # out = x * (c1 + sb*(ps + 0.5*ps^2)) in one fused DVE pass
                oT = midpool.tile([DIM, sz], f32, tag=f"oT{c}")
                nc.vector._custom_dve(
                    dve_op, out=oT[:], in0=pss[c][:], in1=xT[:, cs],
                    s0=sb_t, s1=c1_t, imm2=0.5,
                )
                nc.sync.dma_start(out_ext[:, cs], oT[:])

            sgs = [None] * len(_CHUNKS)
            pss = [None] * len(_CHUNKS)
            for c in range(3):
                emit_ax(c)
            if use_sg:
                for c in range(len(_CHUNKS)):
                    if c >= 3:
                        emit_ax(c)
                    emit_sg(c)
                for c in range(len(_CHUNKS)):
                    emit_ln_mm(c)
                for c in range(len(_CHUNKS)):
                    emit_fused(c)
            else:
                for c in range(3):
                    emit_ln_mm(c)
                emit_fused(0)
                emit_ax(3)
                emit_ln_mm(3)
                emit_fused(1)
                emit_ax(4)
                emit_ln_mm(4)
                emit_fused(2)
                emit_fused(3)
                emit_fused(4)

    nc.finalize()
    return nc


def _get_program(use_sg=False):
    if use_sg not in _PROGRAMS:
        _PROGRAMS[use_sg] = _build_program(use_sg)
    return _PROGRAMS[use_sg]


def _host_inputs(x, W_m, M_m, G):
    """Host-side parameter precompute shared by kernel() and test harness.

    Returns (in_maps, aux); aux["mode"] is "fast" (fluctuation dropped),
    "sg" (exact sign matmul), or "host" (full CPU fallback)."""
    dim = DIM
    eye = np.eye(dim, dtype=np.float32)
    wm = eye + (1.0 - eye) * np.tanh(W_m) * (1.0 / (1.0 + np.exp(-M_m)))
    wm = wm.astype(np.float32)
    a = np.abs(wm)
    one_m_2a = 1.0 - 2.0 * a
    with np.errstate(divide="ignore"):
        L = np.log(np.abs(one_m_2a)).astype(np.float32)
    np.fill_diagonal(L, 0.0)
    g = np.tanh(G).astype(np.float32)

    # --- device-path validity checks (cheap, O(N d + d^2)) ---------------
    off = one_m_2a.copy()
    np.fill_diagonal(off, 1.0)
    sign_ok = bool((off > 0.0).all())

    xbf = x.astype(BF16)
    absx = np.abs(xbf.astype(np.float32))
    eps_ok = bool((absx >= EPS).all())

    max_absx = float(absx.max()) if absx.size else 1.0
    max_lg = np.log(max(max_absx, EPS))
    maxabs_lg = max(abs(np.log(EPS)), abs(max_lg))
    a_off = a - np.diag(np.diag(a))
    s_off = float(a_off.sum(axis=0).max())
    omega_ok = bool(max_lg + maxabs_lg * s_off < OMEGA - 0.25)
    # Taylor validity: |exp argument| bound small enough for 2nd order
    fl_bound = 0.5 * float(np.abs(L).sum(axis=0).max())
    taylor_ok = bool(maxabs_lg * s_off + fl_bound < 0.25)
    # sign-fluctuation term droppable when its relative effect is tiny
    drop_ok = bool(np.expm1(fl_bound) < 5e-3)

    if sign_ok and eps_ok and omega_ok and taylor_ok:
        mode = "fast" if drop_ok else "sg"
    else:
        mode = "host"

    # --- packed constants -------------------------------------------------
    wmi = (wm - eye).astype(BF16)
    wmi2 = (0.5 * (wm - eye)).astype(BF16)
    mlh = (-0.5 * L).astype(BF16)
    colsum = 0.5 * L.sum(axis=0, dtype=np.float64)
    sb = (g.astype(np.float64) * np.exp(colsum)).astype(np.float32)
    c1 = (1.0 + sb).astype(np.float32)
    sb_u16 = sb.view(np.uint16).reshape(dim, 2)
    c1_u16 = c1.view(np.uint16).reshape(dim, 2)

    consts = np.zeros((dim, _C_COLS), dtype=np.uint16)
    consts[:, _C_WMI : _C_WMI + dim] = wmi.view(np.uint16)
    consts[:, _C_WMI2 : _C_WMI2 + dim] = wmi2.view(np.uint16)
    consts[:, _C_MLH : _C_MLH + dim] = mlh.view(np.uint16)
    consts[:, _C_SB] = sb_u16[:, 0]
    consts[:, _C_SB + 1] = sb_u16[:, 1]
    consts[:, _C_C1] = c1_u16[:, 0]
    consts[:, _C_C1 + 1] = c1_u16[:, 1]
    consts_bf = consts.view(BF16)

    in_maps = []
    for cid in range(N_CORES):
        rows = slice(cid * SHARD, (cid + 1) * SHARD)
        in_maps.append(
            {
                "xt": np.ascontiguousarray(xbf[rows].T),
                "consts": consts_bf,
            }
        )

    aux = {"wm": wm, "a": a, "one_m_2a": one_m_2a, "g": g, "mode": mode}
    return in_maps, aux


def kernel(x, W_m, M_m, G):
    from concourse.bass_utils import run_bass_kernel_spmd

    x = np.asarray(x, dtype=np.float32)
    W_m = np.asarray(W_m, dtype=np.float32)
    M_m = np.asarray(M_m, dtype=np.float32)
    G = np.asarray(G, dtype=np.float32)

    in_maps, aux = _host_inputs(x, W_m, M_m, G)

    if aux["mode"] == "host":
        # General-case fixup (never taken for the reference data):
        # compute the output exactly on the host.
        wm, a, one_m_2a, g = aux["wm"], aux["a"], aux["one_m_2a"], aux["g"]
        lg_h = np.log(np.maximum(np.abs(x), EPS))
        ls = lg_h @ wm
        mul = np.exp(np.minimum(ls, OMEGA))
        msv = np.ones_like(x)
        for i in range(DIM):
            f = np.where(
                x[:, i : i + 1] > 0,
                1.0,
                np.where(x[:, i : i + 1] < 0, one_m_2a[i], 1.0 - a[i]),
            )
            msv *= f
        return (x + mul * msv * g).astype(np.float32)

    nc = _get_program(use_sg=(aux["mode"] == "sg"))
    res = run_bass_kernel_spmd(nc, in_maps, core_ids=list(range(N_CORES)))
    out = np.empty((N_TOTAL, DIM), dtype=np.float32)
    for cid, r in enumerate(res.results):
        rows = slice(cid * SHARD, (cid + 1) * SHARD)
        out[rows] = r["out"].T
    return out
